# revision 5
# baseline (speedup 1.0000x reference)
"""Trainium2 Bass kernel for BronxModel (GNN message passing SDE).

Strategy (8 NeuronCores, SPMD):
  - Nodes dst-sharded across cores; within a core, dst nodes sorted by
    in-degree and tiled 128/partition-tile for tight gather-grid padding.
  - State Y kept in SBUF, rescaled by (1-dt)^-k per step so the update is
    Y += rdi_dt*agg + dw (constants folded host-side; W_out rescaled).
  - Per SDE step: per-tile PE matmul Y@W_msg scaled by rsqrt(deg_out) ->
    bf16 message shard; AllGather into a per-step Shared DRAM table;
    per (tile, column) indirect-DMA row gather of the padded message grid;
    DVE tree-reduce over columns (segment sum); fused Y update.
  - Grid padding points at an always-zero pad row inside the table
    (pad positions have Y=0, dW=0, rdi=0 so their messages stay 0).
  - h @ W_in precomputed on host; final Y @ (W_out*(1-dt)^STEPS) on device.
"""
import os
import sys

sys.path.insert(0, "/opt/trn_rl_repo")

import numpy as np
import ml_dtypes

import concourse.bass as bass
import concourse.bacc as bacc
import concourse.tile as tile
import concourse.mybir as mybir
from concourse import bass_utils

NCORES = 8
P = 128


def _preprocess(h, W_in, W_msg, W_out, dW, src, dst):
    """Host-side graph partitioning + grid construction. Returns per-core
    input maps and metadata needed to build the bass program."""
    N = h.shape[0]
    E = src.shape[0]
    HID = W_msg.shape[0]
    STEPS = dW.shape[0]
    OUTF = W_out.shape[1]
    DT = 1.0 / STEPS
    SIGMA = 0.01
    sqrt_dt = np.sqrt(DT).astype(np.float32)

    nsh = (N + NCORES - 1) // NCORES          # nodes per core (last may be short)
    T = (nsh + P - 1) // P                     # tiles per core
    npad = T * P                               # padded nodes per core

    ones = np.ones(E, np.float32)
    deg_out = np.zeros(N, np.float32)
    np.add.at(deg_out, src, ones)
    deg_in = np.zeros(N, np.float32)
    np.add.at(deg_in, dst, ones)
    rdo = 1.0 / np.sqrt(np.maximum(deg_out, 1.0))
    rdi = 1.0 / np.sqrt(np.maximum(deg_in, 1.0))

    x0 = (h.astype(np.float32) @ W_in.astype(np.float32)).astype(np.float32)

    # per-core orderings (sorted ascending by in-degree), table row mapping
    ords = []          # ords[c][i] = global node at position i of core c
    pos_of = np.zeros(N, np.int64)
    for c in range(NCORES):
        lo, hi = c * nsh, min((c + 1) * nsh, N)
        nodes = np.arange(lo, hi)
        o = nodes[np.argsort(deg_in[nodes], kind="stable")]
        ords.append(o)
        pos_of[o] = np.arange(len(o))
    # table row of node n (staged layout [128, T*HID]): row = c*npad + p*T + t
    n_owner = np.minimum(np.arange(N) // nsh, NCORES - 1)
    n_pos = pos_of  # position i within core
    n_t = n_pos // P
    n_p = n_pos % P
    table_row = n_owner * npad + n_p * T + n_t   # int64
    TBL = NCORES * npad
    ZROW = npad - 1  # core-0 pad position (always-zero message row)

    # per-core grids
    e_owner = np.minimum(dst // nsh, NCORES - 1)
    grids = []
    Lts_all = []
    for c in range(NCORES):
        m = e_owner == c
        s_c = src[m]
        d_c = dst[m]
        pos = pos_of[d_c]                        # position of dst within core
        order = np.argsort(pos, kind="stable")
        s_c = s_c[order]
        pos = pos[order]
        # per-position counts
        cnt = np.zeros(npad, np.int64)
        np.add.at(cnt, pos, 1)
        # L per tile
        Lts = cnt.reshape(T, P).max(axis=1)
        Lts_all.append(Lts)
        grids.append((s_c, pos, cnt))
    Lts = np.maximum.reduce(Lts_all)             # shared across cores
    Lts = np.maximum(Lts, 1)
    CT = int(Lts.sum())
    col_base = np.concatenate([[0], np.cumsum(Lts)[:-1]]).astype(np.int64)

    # fold (1-dt)^-k scaling: Y_k = y_k / s_k, s_k = (1-dt)^k
    s = (1.0 - DT) ** np.arange(STEPS + 1)

    in_maps = []
    dWs = dW.astype(np.float32) * (SIGMA * sqrt_dt)
    dWs = dWs / s[1:, None, None].astype(np.float32)  # per-step 1/s_{k+1}
    for c in range(NCORES):
        s_c, pos, cnt = grids[c]
        grid = np.full((P, CT), ZROW, np.int32)
        # slot within each dst's list
        slot = np.zeros(len(pos), np.int64)
        if len(pos):
            # pos sorted ascending; slot = index within equal-pos run
            first = np.concatenate([[True], pos[1:] != pos[:-1]])
            idx = np.arange(len(pos))
            start = np.maximum.accumulate(np.where(first, idx, 0))
            slot = idx - start
        t_arr = pos // P
        p_arr = pos % P
        cols = col_base[t_arr] + slot
        grid[p_arr, cols] = table_row[s_c].astype(np.int32)

        o = ords[c]
        nreal = len(o)
        # y0 swizzled [P, T*HID]
        y0 = np.zeros((P, T * HID), np.float32)
        ytmp = np.zeros((npad, HID), np.float32)
        ytmp[:nreal] = x0[o]
        y0v = ytmp.reshape(T, P, HID)
        for t in range(T):
            y0[:, t * HID : (t + 1) * HID] = y0v[t]
        # rdi*dt/(1-dt) full [P, T*HID]; zero at pad positions
        rv = np.zeros(npad, np.float32)
        rv[:nreal] = rdi[o] * DT / (1.0 - DT)
        rdi_full = np.repeat(rv.reshape(T, P), HID, axis=1).reshape(T, P, HID)
        rdi_full = np.concatenate([rdi_full[t] for t in range(T)], axis=1)
        # rdo per position [P, T]
        ro = np.zeros(npad, np.float32)
        ro[:nreal] = rdo[o]
        rdo_col = ro.reshape(T, P).T.copy()
        # dW swizzled [STEPS, P, T*HID] in bf16
        dwc = np.zeros((STEPS, P, T * HID), np.float32)
        dtmp = np.zeros((STEPS, npad, HID), np.float32)
        dtmp[:, :nreal] = dWs[:, o, :]
        dv = dtmp.reshape(STEPS, T, P, HID)
        for t in range(T):
            dwc[:, :, t * HID : (t + 1) * HID] = dv[:, t]
        in_maps.append(
            {
                "y0_in": y0,
                "dw_in": dwc.astype(ml_dtypes.bfloat16),
                "rdi_in": rdi_full,
                "rdo_in": rdo_col,
                "grid_in": grid,
                "wmsg_in": W_msg.astype(np.float32),
                "wout_in": (W_out.astype(np.float32) * s[STEPS]),
            }
        )

    meta = dict(
        N=N, HID=HID, OUTF=OUTF, STEPS=STEPS, DT=DT, T=T, npad=npad,
        TBL=TBL, CT=CT, Lts=[int(x) for x in Lts],
        col_base=[int(x) for x in col_base], ords=ords, nsh=nsh,
    )
    return in_maps, meta


def _build(meta, steps_mult=1):
    from concourse.masks import make_identity

    HID, OUTF, STEPS = meta["HID"], meta["OUTF"], meta["STEPS"]
    T, TBL, CT, Lts = meta["T"], meta["TBL"], meta["CT"], meta["Lts"]
    col_base = meta["col_base"]
    W = T * HID
    NSTEP = STEPS * steps_mult

    nc = bacc.Bacc("TRN2", target_bir_lowering=False, debug=False,
                   num_devices=NCORES)
    y0_in = nc.dram_tensor("y0_in", [P, W], mybir.dt.float32, kind="ExternalInput")
    dw_in = nc.dram_tensor("dw_in", [STEPS, P, W], mybir.dt.bfloat16, kind="ExternalInput")
    rdi_in = nc.dram_tensor("rdi_in", [P, W], mybir.dt.float32, kind="ExternalInput")
    rdo_in = nc.dram_tensor("rdo_in", [P, T], mybir.dt.float32, kind="ExternalInput")
    grid_in = nc.dram_tensor("grid_in", [P, CT], mybir.dt.int32, kind="ExternalInput")
    wmsg_in = nc.dram_tensor("wmsg_in", [HID, HID], mybir.dt.float32, kind="ExternalInput")
    wout_in = nc.dram_tensor("wout_in", [HID, OUTF], mybir.dt.float32, kind="ExternalInput")
    out_d = nc.dram_tensor("out_d", [P, T * OUTF], mybir.dt.float32, kind="ExternalOutput")

    with tile.TileContext(nc) as tc:
        with (
            tc.tile_pool(name="state", bufs=1) as st,
            tc.tile_pool(name="work", bufs=4) as wk,
            tc.tile_pool(name="dwp", bufs=2) as dwp,
            tc.tile_pool(name="psum", bufs=2, space="PSUM") as ps,
            tc.tile_pool(name="dram", bufs=1, space="DRAM") as dram,
        ):
            y = st.tile([P, W], mybir.dt.float32)
            nc.sync.dma_start(y[:], y0_in[:, :])
            rdi_t = st.tile([P, W], mybir.dt.float32)
            nc.sync.dma_start(rdi_t[:], rdi_in[:, :])
            rdo_t = st.tile([P, T], mybir.dt.float32)
            nc.sync.dma_start(rdo_t[:], rdo_in[:, :])
            grid_t = st.tile([P, CT], mybir.dt.int32)
            nc.sync.dma_start(grid_t[:], grid_in[:, :])
            wmsg = st.tile([HID, HID], mybir.dt.float32)
            nc.sync.dma_start(wmsg[:], wmsg_in[:, :])
            wout = st.tile([HID, OUTF], mybir.dt.float32)
            nc.sync.dma_start(wout[:], wout_in[:, :])
            ident = st.tile([P, P], mybir.dt.float32)
            make_identity(nc, ident[:])
            MSG_DT = mybir.dt.float8e4
            m_stage = st.tile([P, W], MSG_DT)
            agg = st.tile([P, W], mybir.dt.float32)

            tables = [
                dram.tile([TBL, HID], MSG_DT,
                          addr_space="Shared", name=f"table{k}")
                for k in range(NSTEP)
            ]
            bounce = dram.tile([P, W], MSG_DT)

            for k in range(NSTEP):
                table = tables[k]
                dwk = dwp.tile([P, W], mybir.dt.bfloat16, tag="dw")
                nc.sync.dma_start(dwk[:], dw_in[k % STEPS, :, :])
                # messages: m = (Y @ Wmsg) * rdo  -> bf16 staged
                for t in range(T):
                    ytp = ps.tile([HID, P], mybir.dt.float32, space="PSUM", tag="ytp")
                    nc.tensor.transpose(
                        out=ytp[:], in_=y[:, t * HID : (t + 1) * HID], identity=ident[:]
                    )
                    yT = wk.tile([HID, P], mybir.dt.float32, tag="yT")
                    nc.scalar.activation(
                        yT[:], ytp[:], mybir.ActivationFunctionType.Copy
                    )
                    mp = ps.tile([P, HID], mybir.dt.float32, space="PSUM", tag="mp")
                    nc.tensor.matmul(
                        out=mp[:], lhsT=yT[:], rhs=wmsg[:], start=True, stop=True
                    )
                    nc.scalar.activation(
                        m_stage[:, t * HID : (t + 1) * HID],
                        mp[:],
                        mybir.ActivationFunctionType.Copy,
                        scale=rdo_t[:, t : t + 1],
                    )
                nc.sync.dma_start(bounce[:], m_stage[:])
                nc.gpsimd.collective_compute(
                    "AllGather",
                    mybir.AluOpType.bypass,
                    replica_groups=[list(range(NCORES))],
                    ins=[bounce[:]],
                    outs=[table[:, :]],
                )
                # gather + segment sum per tile
                for t in range(T):
                    L = Lts[t]
                    cb = col_base[t]
                    msgs = wk.tile([P, L * HID], MSG_DT, tag="msgs")
                    for l in range(L):
                        nc.gpsimd.indirect_dma_start(
                            out=msgs[:, l * HID : (l + 1) * HID],
                            out_offset=None,
                            in_=table[:, :],
                            in_offset=bass.IndirectOffsetOnAxis(
                                ap=grid_t[:, cb + l : cb + l + 1], axis=0
                            ),
                        )
                    # tree reduce over L columns into agg slice (f32)
                    aslice = agg[:, t * HID : (t + 1) * HID]
                    if L == 1:
                        nc.vector.tensor_copy(aslice, msgs[:, 0:HID])
                    else:
                        half = L // 2
                        rem = L - half
                        sc = wk.tile([P, rem * HID], mybir.dt.float32, tag="sc")
                        nc.vector.tensor_add(
                            out=sc[:, 0 : half * HID],
                            in0=msgs[:, 0 : half * HID],
                            in1=msgs[:, rem * HID : L * HID],
                        )
                        if rem > half:
                            nc.vector.tensor_copy(
                                sc[:, half * HID : rem * HID],
                                msgs[:, half * HID : rem * HID],
                            )
                        span = rem
                        while span > 1:
                            h2 = span // 2
                            r2 = span - h2
                            nc.vector.tensor_add(
                                out=sc[:, 0 : h2 * HID],
                                in0=sc[:, 0 : h2 * HID],
                                in1=sc[:, r2 * HID : span * HID],
                            )
                            span = r2
                        nc.vector.tensor_copy(aslice, sc[:, 0:HID])
                # Y update: Y += agg*rdi_dt + dw   (all scalings folded)
                nc.vector.tensor_mul(agg[:], agg[:], rdi_t[:])
                nc.vector.tensor_add(y[:], y[:], agg[:])
                nc.vector.tensor_add(y[:], y[:], dwk[:])

            out_stage = st.tile([P, T * OUTF], mybir.dt.float32)
            for t in range(T):
                ytp = ps.tile([HID, P], mybir.dt.float32, space="PSUM", tag="ytp")
                nc.tensor.transpose(
                    out=ytp[:], in_=y[:, t * HID : (t + 1) * HID], identity=ident[:]
                )
                yT = wk.tile([HID, P], mybir.dt.float32, tag="yT")
                nc.scalar.activation(yT[:], ytp[:], mybir.ActivationFunctionType.Copy)
                op = ps.tile([P, OUTF], mybir.dt.float32, space="PSUM", tag="op")
                nc.tensor.matmul(out=op[:], lhsT=yT[:], rhs=wout[:], start=True, stop=True)
                nc.scalar.activation(
                    out_stage[:, t * OUTF : (t + 1) * OUTF],
                    op[:],
                    mybir.ActivationFunctionType.Copy,
                )
            nc.sync.dma_start(out_d[:, :], out_stage[:])

    nc.compile()
    return nc


def _build_scaled(meta, steps_mult):
    return _build(meta, steps_mult=steps_mult)


def kernel(h, W_in, W_msg, W_out, dW, src, dst):
    h = np.asarray(h)
    W_in = np.asarray(W_in)
    W_msg = np.asarray(W_msg)
    W_out = np.asarray(W_out)
    dW = np.asarray(dW)
    src = np.asarray(src)
    dst = np.asarray(dst)

    in_maps, meta = _preprocess(h, W_in, W_msg, W_out, dW, src, dst)
    nc = _build(meta)

    res = bass_utils.run_bass_kernel_spmd(
        nc, in_maps, core_ids=list(range(NCORES)), trace=False
    )

    N, OUTF, T = meta["N"], meta["OUTF"], meta["T"]
    npad, nsh = meta["npad"], meta["nsh"]
    out = np.zeros((N, OUTF), np.float32)
    for c in range(NCORES):
        o = meta["ords"][c]
        dev = res.results[c]["out_d"]  # [P, T*OUTF]
        dev = dev.reshape(P, T, OUTF).transpose(1, 0, 2).reshape(npad, OUTF)
        out[o] = dev[: len(o)]
    return out
